# revision 1
# baseline (speedup 1.0000x reference)
"""Multi-head causal self-attention forward on 8 Trainium2 NeuronCores.

Problem: x[4,2048,1024] @ w_qkv[1024,3072] -> causal MHA (16 heads, d=64)
         -> @ w_out[1024,1024] + b_out.

Sharding: core c handles batch b = c//2 and head-group g = c%2 (8 heads).
Each core computes a partial output  attn_out_heads(g) @ w_out[rows(g)]
for its batch; host sums the two partials per batch (row-parallel out
projection) and adds b_out.

Per-core kernel (bf16 matmul inputs, fp32 PSUM accumulate):
  - V = (xT as lhsT) @ w_v, stored [tokens,dv] with an interleaved
    ones-column per head ([v|1] width 65 -> fused softmax denominator).
  - per head-pair m: QT,KT = (w_q/w_k as lhsT) @ xT  (head dims on
    partitions: pair half A in partitions 0..63, half B in 64..127),
    then causal attention for both heads with ST matmuls row-packed
    into disjoint PE array row groups (A: rows 0-63, B: rows 64-127).
  - attention in [k,q] orientation: S^T = kT.T @ qT per (k-chunk 128,
    q-group 1024); exp on ScalarE with scale=1/8 folded in; causal via
    block skip + memset + triangular-mask multiply on the diagonal
    block; PV: u[65,512] += [v|1].T @ P accumulated over k-chunks.
  - normalize with reciprocal_approx_fast (SBUF) + gpsimd
    partition_broadcast + DVE multiply -> hd tiles [hd, q] bf16.
  - out = hd.T @ w_out_shard accumulated over 4 hd chunks -> y f32.
"""

import sys

sys.path.insert(0, "/opt/trn_rl_repo")

import numpy as np
import ml_dtypes

import concourse.bass as bass
import concourse.tile as tile
from concourse import bacc, mybir
from concourse.bass_utils import run_bass_kernel_spmd

BF16 = mybir.dt.bfloat16
F32 = mybir.dt.float32
NP_BF16 = ml_dtypes.bfloat16
EXP = mybir.ActivationFunctionType.Exp

B, T, C = 4, 2048, 1024
NCORES = 8
HC = 8  # heads per core
D = 64
DQ = HC * D  # 512
CA = C // 128  # 8 contraction chunks
NT128 = T // 128  # 16
NT512 = T // 512  # 4
SCALE = 1.0 / 8.0

_cached = None


def _build():
    nc = bacc.Bacc("TRN2", target_bir_lowering=False, debug=False, num_devices=NCORES)

    xT = nc.dram_tensor("xT", [C, T], BF16, kind="ExternalInput")
    wq = nc.dram_tensor("wq", [C, DQ], BF16, kind="ExternalInput")
    wk = nc.dram_tensor("wk", [C, DQ], BF16, kind="ExternalInput")
    wv = nc.dram_tensor("wv", [C, DQ], BF16, kind="ExternalInput")
    wo = nc.dram_tensor("wo", [DQ, C], BF16, kind="ExternalInput")
    trid = nc.dram_tensor("tri", [128, 128], BF16, kind="ExternalInput")
    y = nc.dram_tensor("y", [T, C], F32, kind="ExternalOutput")

    with tile.TileContext(nc) as tc:
        _emit(tc, nc, xT, wq, wk, wv, wo, trid, y)
    nc.compile()
    return nc


def _emit(tc, nc, xT, wq, wk, wv, wo, trid, y):
    from contextlib import ExitStack

    with ExitStack() as ctx:
        ep = ctx.enter_context

        persist = ep(tc.tile_pool(name="persist", bufs=1))
        qts = [persist.tile([128, T], BF16, tag=f"qt{m}", name=f"qt{m}") for m in range(4)]
        kts = [persist.tile([128, T], BF16, tag=f"kt{m}", name=f"kt{m}") for m in range(4)]
        vts = [persist.tile([128, HC * 65], BF16, tag=f"v{i}", name=f"v{i}") for i in range(NT128)]
        hds = [persist.tile([128, T], BF16, tag=f"hd{j}", name=f"hd{j}") for j in range(4)]
        wo_sb = [persist.tile([128, C], BF16, tag=f"wo{j}", name=f"wo{j}") for j in range(4)]
        tri = persist.tile([128, 128], BF16, tag="tri", name="tri")

        xin = ep(tc.tile_pool(name="xin", bufs=1))
        xts = [xin.tile([128, T], BF16, tag=f"x{a}", name=f"x{a}") for a in range(CA)]
        wq_sb = [xin.tile([128, DQ], BF16, tag=f"wq{a}", name=f"wqs{a}") for a in range(CA)]
        wk_sb = [xin.tile([128, DQ], BF16, tag=f"wk{a}", name=f"wks{a}") for a in range(CA)]
        wv_sb = [xin.tile([128, DQ], BF16, tag=f"wv{a}", name=f"wvs{a}") for a in range(CA)]

        # PSUM: "stps" shared by V/QKV/ST ([128,1024] f32 slots = 2 banks
        # x 2 bufs) + "u" PV accumulators (1 bank x 4 bufs) = 8 banks.
        stp = ep(tc.tile_pool(name="stp", bufs=2, space="PSUM"))
        u_ps = ep(tc.tile_pool(name="u_ps", bufs=4, space="PSUM"))
        p_pool = ep(tc.tile_pool(name="p_pool", bufs=12))
        norm = ep(tc.tile_pool(name="norm", bufs=4))

        # V path (x + w_v) first so the opening matmuls are not DMA-gated,
        # then q/k weights, then late-use tensors.
        for a in range(CA):
            sl = slice(a * 128, (a + 1) * 128)
            nc.sync.dma_start(out=xts[a], in_=xT[sl, :])
            nc.sync.dma_start(out=wv_sb[a], in_=wv[sl, :])
        for a in range(CA):
            sl = slice(a * 128, (a + 1) * 128)
            nc.sync.dma_start(out=wq_sb[a], in_=wq[sl, :])
            nc.sync.dma_start(out=wk_sb[a], in_=wk[sl, :])
        nc.sync.dma_start(out=tri, in_=trid[:, :])
        for j in range(4):
            nc.sync.dma_start(out=wo_sb[j], in_=wo[j * 128 : (j + 1) * 128, :])

        def emit_v(tk):
            ps = stp.tile([128, 512], F32, tag="stps", name="vps")
            for a in range(CA):
                nc.tensor.matmul(
                    ps,
                    xts[a][:, tk * 128 : (tk + 1) * 128],
                    wv_sb[a],
                    start=(a == 0),
                    stop=(a == CA - 1),
                )
            v_view = vts[tk].rearrange("p (h e) -> p h e", e=65)
            nc.vector.tensor_copy(
                v_view[:, :, 0:64], ps.rearrange("p (h e) -> p h e", e=64)
            )
            nc.vector.memset(v_view[:, :, 64:65], 1.0)

        def emit_qkt(m, wsb, dst):
            for tbp in range(2):  # token blocks of 1024
                ps = stp.tile([128, 1024], F32, tag="stps", name="qkps")
                for a in range(CA):
                    lhsT = wsb[a][:, m * 128 : (m + 1) * 128]
                    for hb in range(2):
                        nc.tensor.matmul(
                            ps[:, hb * 512 : hb * 512 + 512],
                            lhsT,
                            xts[a][
                                :,
                                tbp * 1024 + hb * 512 : tbp * 1024 + hb * 512 + 512,
                            ],
                            start=(a == 0),
                            stop=(a == CA - 1),
                        )
                nc.vector.tensor_copy(dst[m][:, tbp * 1024 : tbp * 1024 + 1024], ps)

        def emit_st_exp(m, qg, ik, half):
            qlo = 1024 * qg
            kc = slice(ik * 128, (ik + 1) * 128)
            c0 = max(0, 128 * ik - qlo)
            rq = slice(half * 64, half * 64 + 64)
            s_ps = stp.tile([128, 1024], F32, tag="stps", name="sps")
            p_t = p_pool.tile([128, 1024], BF16, tag="p", name="pt")
            for qb in (2 * qg, 2 * qg + 1):
                if 4 * qb + 3 < ik:
                    continue
                lo = max(qb * 512, qlo + c0)  # global q start
                n = (qb + 1) * 512 - lo
                nc.tensor.matmul(
                    s_ps[:, lo - qlo : lo - qlo + n],
                    kts[m][rq, kc],
                    qts[m][rq, lo : lo + n],
                    start=True,
                    stop=True,
                )
            nc.scalar.activation(p_t[:, c0:1024], s_ps[:, c0:1024], EXP, scale=SCALE)
            if 128 * ik >= qlo:  # diagonal block: multiplicative causal mask
                nc.vector.tensor_mul(p_t[:, c0 : c0 + 128], p_t[:, c0 : c0 + 128], tri)
            return p_t

        def emit_pv_norm(m, qg, ik, half, p_t, us):
            qlo = 1024 * qg
            c0 = max(0, 128 * ik - qlo)
            h = 2 * m + half
            rq = slice(half * 64, half * 64 + 64)
            v_lhsT = vts[ik][:, h * 65 : h * 65 + 65]
            for qb in (2 * qg, 2 * qg + 1):
                if 4 * qb + 3 < ik:
                    continue
                a0 = qb * 512 - qlo
                off = max(0, c0 - a0)  # clip masked cols
                nc.tensor.matmul(
                    us[(half, qb)][:, off:512],
                    v_lhsT,
                    p_t[:, a0 + off : a0 + 512],
                    start=(ik == 0),
                    stop=(ik == 4 * qb + 3),
                    skip_group_check=True,
                )
                if ik == 4 * qb + 3:
                    # u complete: normalize into hd tiles
                    u = us[(half, qb)]
                    rec_in = norm.tile([1, 512], F32, tag="ri", name="ri")
                    nc.vector.tensor_copy(rec_in, u[64:65, :])
                    rec = norm.tile([1, 512], F32, tag="rc", name="rc")
                    nc.vector.reciprocal_approx_fast(out=rec, in_=rec_in)
                    bc = norm.tile([64, 512], F32, tag="bc", name="bc")
                    nc.gpsimd.partition_broadcast(bc, rec)
                    nc.vector.tensor_mul(
                        hds[m][rq, qb * 512 : (qb + 1) * 512], u[0:64, :], bc
                    )

        LAG = 4  # ST/exp stream runs this many (ik,half) steps ahead of PV

        def attn_sub(m, qg, iks, us):
            """Software-pipelined ST/exp -> PV over steps (ik, half)."""
            steps = [(ik, half) for ik in iks for half in range(2)]
            staged = {}
            for t in range(len(steps) + LAG):
                if t < len(steps):
                    ik, half = steps[t]
                    staged[(ik, half)] = emit_st_exp(m, qg, ik, half)
                if t == LAG and not us:
                    for half in range(2):
                        for qb in (2 * qg, 2 * qg + 1):
                            us[(half, qb)] = u_ps.tile(
                                [65, 512], F32, tag="u", name=f"u{half}_{qb}"
                            )
                if t >= LAG:
                    ik, half = steps[t - LAG]
                    emit_pv_norm(m, qg, ik, half, staged.pop((ik, half)), us)

        # ---- per head-pair: QT/KT projection, then attention ----
        # V chunks and the next pair's Q/K projections are interleaved at
        # attention boundaries so PE and ScalarE both stay fed.
        emit_qkt(0, wq_sb, qts)
        emit_qkt(0, wk_sb, kts)
        for m in range(4):
            if m == 0:
                for tk in range(8):
                    emit_v(tk)
            us0 = {}
            attn_sub(m, 0, range(8), us0)
            if m == 0:
                for tk in range(8, 16):
                    emit_v(tk)
            if m < 3:
                emit_qkt(m + 1, wq_sb, qts)
            us1 = {}
            attn_sub(m, 1, range(8), us1)
            if m < 3:
                emit_qkt(m + 1, wk_sb, kts)
            attn_sub(m, 1, range(8, 16), us1)

        # ---- output projection ----
        for tq in range(NT128):
            pss = [
                u_ps.tile([128, 512], F32, tag="u", name="projps") for _ in range(2)
            ]
            for j in range(4):
                lhsT = hds[j][:, tq * 128 : (tq + 1) * 128]
                for nb in range(2):
                    nc.tensor.matmul(
                        pss[nb],
                        lhsT,
                        wo_sb[j][:, nb * 512 : (nb + 1) * 512],
                        start=(j == 0),
                        stop=(j == 3),
                    )
            for nb in range(2):
                ob = norm.tile([128, 512], F32, tag="ob", name="ob")
                nc.scalar.copy(ob, pss[nb])
                nc.sync.dma_start(
                    out=y[tq * 128 : (tq + 1) * 128, nb * 512 : (nb + 1) * 512],
                    in_=ob,
                )


def _in_maps(x, w_qkv, w_out):
    maps = []
    for c in range(NCORES):
        b, g = c // 2, c % 2
        h0 = g * DQ
        maps.append(
            {
                "xT": np.ascontiguousarray(x[b].T).astype(NP_BF16),
                "wq": w_qkv[:, h0 : h0 + DQ].astype(NP_BF16),
                "wk": w_qkv[:, C + h0 : C + h0 + DQ].astype(NP_BF16),
                "wv": w_qkv[:, 2 * C + h0 : 2 * C + h0 + DQ].astype(NP_BF16),
                "wo": np.ascontiguousarray(w_out[h0 : h0 + DQ, :]).astype(NP_BF16),
                "tri": np.triu(np.ones((128, 128), dtype=np.float32)).astype(NP_BF16),
            }
        )
    return maps


def get_bass():
    global _cached
    if _cached is None:
        _cached = _build()
    return _cached


def run(x, w_qkv, w_out, b_out, **spmd_kwargs):
    nc = get_bass()
    res = run_bass_kernel_spmd(
        nc, _in_maps(x, w_qkv, w_out), core_ids=list(range(NCORES)), **spmd_kwargs
    )
    out = np.empty((B, T, C), dtype=np.float32)
    for b in range(B):
        out[b] = res.results[2 * b]["y"] + res.results[2 * b + 1]["y"]
    out += b_out.astype(np.float32)
    return out, res


def kernel(x, w_qkv, w_out, b_out):
    x = np.asarray(x)
    w_qkv = np.asarray(w_qkv)
    w_out = np.asarray(w_out)
    b_out = np.asarray(b_out)
    out, _ = run(x, w_qkv, w_out, b_out)
    return out

